# revision 1
# baseline (speedup 1.0000x reference)
"""Local (banded, window=3) attention TRN2 kernel.

Full-input contract: kernel(**inputs) takes the complete tensors
  x [8, 1024, 384], qkv_w [1152, 384], proj_w [384, 384], proj_b [384]
and returns the full output [8, 1024, 384].

Sharding: data-parallel over batch B=8 -> one batch element per NeuronCore.

Per-core algorithm (bf16 data, fp32 PSUM accumulation, fp32 softmax):
  xT [C=384, N=1024] (host-pretransposed shard, bf16)
  qkvT[ch, t] = qkv_w @ x_b.T          (PE; lhsT = host-pretransposed qkv_w.T)
  band scores s_off[h, t] = sum_d q[(h,d),t] * k[(h,d),t+off], off in {-1,0,+1}
     products on DVE in [ch, t] layout (token shift = free-dim slice),
     partition-reduction over d via PE matmul against a 0/1 head-indicator
  p = softmax over the 3 offsets (ACT exp, DVE add / recip-approx / mul)
  attn_outT[(h,d), t] = sum_off pbcast_off[(h,d), t] * vT[(h,d), t+off]
     (p broadcast head->64 rows via PE indicator matmul into PSUM,
      multiply-add on DVE reading PSUM directly)
  yT = proj_w @ attn_outT + b     (PE; bias folded in as a K=1 matmul)
Host transposes yT back to [1024, 384] fp32 per batch element.
"""

import numpy as np

B, N, C = 8, 1024, 384
H, HD = 6, 64
CQKV = 3 * C  # 1152
NCORES = 8
P = 128
NHALF = N // 2  # 512
KC = C // P  # 3 contraction chunks

_cached = {}


def _build_nc():
    import contextlib

    import concourse.bacc as bacc
    import concourse.tile as tile
    from concourse import mybir

    f32 = mybir.dt.float32
    bf16 = mybir.dt.bfloat16
    AF = mybir.ActivationFunctionType

    nc = bacc.Bacc("TRN2", target_bir_lowering=False, debug=False,
                   num_devices=NCORES)

    d_xT = nc.dram_tensor("xT", [C, N], bf16, kind="ExternalInput").ap()
    d_wqkvT = nc.dram_tensor("qkv_wT", [C, CQKV], bf16,
                             kind="ExternalInput").ap()
    d_wprojT = nc.dram_tensor("proj_wT", [C, C], bf16,
                              kind="ExternalInput").ap()
    d_bias = nc.dram_tensor("proj_b", [1, C], bf16, kind="ExternalInput").ap()
    d_ind6 = nc.dram_tensor("ind6", [P, 6 * KC], bf16,
                            kind="ExternalInput").ap()
    d_ind6T = nc.dram_tensor("ind6T", [H, P * KC], bf16,
                             kind="ExternalInput").ap()
    d_ones = nc.dram_tensor("ones", [1, N], bf16, kind="ExternalInput").ap()
    d_yT = nc.dram_tensor("yT", [C, N], bf16, kind="ExternalOutput").ap()

    with tile.TileContext(nc) as tc, contextlib.ExitStack() as ctx:
        wpool = ctx.enter_context(tc.tile_pool(name="w", bufs=1))
        xpool = ctx.enter_context(tc.tile_pool(name="x", bufs=1))
        qkvpool = ctx.enter_context(tc.tile_pool(name="qkv", bufs=1))
        prodpool = ctx.enter_context(tc.tile_pool(name="prod", bufs=12))
        avpool = ctx.enter_context(tc.tile_pool(name="av", bufs=10))
        aopool = ctx.enter_context(tc.tile_pool(name="ao", bufs=1))
        ypool = ctx.enter_context(tc.tile_pool(name="y", bufs=4))
        epool = ctx.enter_context(tc.tile_pool(name="e", bufs=20))
        # PSUM budget (8 banks of 512 fp32):
        #   mm   [128, 512] = 1 bank x 4 bufs = 4  (stage-1 qkv)
        #   pb   [128, 512]  = 1 bank  x 2 bufs = 2  (p-broadcast + proj)
        #   s    [6, 512]    = 1 bank  x 2 bufs = 2  (scores)
        mmpool = ctx.enter_context(
            tc.tile_pool(name="mm", bufs=4, space="PSUM"))
        pbpool = ctx.enter_context(
            tc.tile_pool(name="pb", bufs=2, space="PSUM"))
        spool = ctx.enter_context(
            tc.tile_pool(name="s", bufs=2, space="PSUM"))

        # ---- inputs: per-chunk DMAs on both HWDGE queues (sync + scalar)
        # so stage-1 matmuls of chunk kc can start as soon as x[kc]/w[kc] land
        w_qkv, w_proj, x_t = [], [], []
        for kc in range(KC):
            xt = xpool.tile([P, N], bf16, name=f"xT{kc}")
            if kc == 0:
                # split so the first stage-1 matmul group starts sooner
                nc.sync.dma_start(out=xt[:, 0:NHALF],
                                  in_=d_xT[0:P, 0:NHALF])
                nc.sync.dma_start(out=xt[:, NHALF:N],
                                  in_=d_xT[0:P, NHALF:N])
            else:
                nc.sync.dma_start(out=xt, in_=d_xT[P * kc:P * (kc + 1), :])
            x_t.append(xt)
            wt = wpool.tile([P, CQKV], bf16, name=f"wqkv{kc}")
            # split per q/k/v part so the first stage-1 matmuls (q chunks)
            # start as soon as the q-part of the weights lands
            for part in range(3):
                nc.scalar.dma_start(
                    out=wt[:, C * part:C * (part + 1)],
                    in_=d_wqkvT[P * kc:P * (kc + 1), C * part:C * (part + 1)])
            w_qkv.append(wt)
        ind6 = wpool.tile([P, 6 * KC], bf16, name="ind6")
        nc.sync.dma_start(out=ind6, in_=d_ind6)
        ind6T = wpool.tile([H, P * KC], bf16, name="ind6T")
        nc.sync.dma_start(out=ind6T, in_=d_ind6T)
        for kc in range(KC):
            pt = wpool.tile([P, C], bf16, name=f"wproj{kc}")
            nc.scalar.dma_start(out=pt, in_=d_wprojT[P * kc:P * (kc + 1), :])
            w_proj.append(pt)
        bias = wpool.tile([1, C], bf16, name="bias")
        nc.sync.dma_start(out=bias, in_=d_bias)
        ones = wpool.tile([1, N], bf16, name="ones")
        nc.sync.dma_start(out=ones, in_=d_ones)

        def stage1_chunk(m, evac_engine):
            """qkvT[m] [128,1024] bf16 = (qkv_w @ x.T) rows 128m..128m+127."""
            qt = qkvpool.tile([P, N], bf16, name=f"qkvT{m}")
            for h in range(2):
                ps = mmpool.tile([P, NHALF], f32, tag="mm")
                for kc in range(KC):
                    nc.tensor.matmul(
                        ps,
                        lhsT=w_qkv[kc][:, P * m:P * (m + 1)],
                        rhs=x_t[kc][:, NHALF * h:NHALF * (h + 1)],
                        start=(kc == 0), stop=(kc == KC - 1),
                    )
                dst = qt[:, NHALF * h:NHALF * (h + 1)]
                nc.scalar.copy(dst, ps)
            return qt

        # ---- stage 1: all 9 qkvT chunks (PE warms up on a dense stream) ----
        # q/k interleaved so prods of chunk kc can start after 2 chunks;
        # v chunks are emitted later (after the score matmuls) so the PE has
        # work queued while the softmax chain runs on ACT/DVE
        qkvT = [None] * 9
        for m in (0, 3, 1, 4, 2, 5):
            qkvT[m] = stage1_chunk(m, "act")

        def half(ap, h):
            return ap[:, NHALF * h:NHALF * (h + 1)]


        # ---- banded attention mid-section, pipelined as 2 half-lanes ----
        # offsets: 0 -> key j=t-1, 1 -> j=t, 2 -> j=t+1
        def make_prod(off, kc):
            """prod[off][kc] [128, 1024] = q * shifted k (DVE, bf16)."""
            q = qkvT[kc]
            k = qkvT[3 + kc]
            pr = prodpool.tile([P, N], bf16, tag="prod",
                               name=f"prod{off}_{kc}")
            if off == 0:
                # col 0 unwritten: masked after exp via e[0] col 0
                nc.vector.tensor_mul(pr[:, 1:], q[:, 1:], k[:, 0:N - 1])
            elif off == 1:
                nc.vector.tensor_mul(pr, q, k)
            else:
                # col N-1 unwritten: masked after exp via e[2] col N-1
                nc.vector.tensor_mul(pr[:, 0:N - 1], q[:, 0:N - 1], k[:, 1:N])
            return pr

        prods = [[make_prod(off, kc) for kc in range(KC)] for off in range(3)]
        e_half = [[None] * 3 for _ in range(2)]   # [h][off]
        for h in range(2):
            for off in range(3):
                sps = spool.tile([H, NHALF], f32, tag="s")
                for kc in range(KC):
                    nc.tensor.matmul(
                        sps,
                        lhsT=ind6[:, 6 * kc:6 * (kc + 1)],
                        rhs=prods[off][kc][:, NHALF * h:NHALF * (h + 1)],
                        start=(kc == 0), stop=(kc == KC - 1),
                    )
                et = epool.tile([H, NHALF], f32, tag="e", name=f"e{h}_{off}")
                with tc.high_priority():
                    nc.scalar.activation(et, sps, AF.Exp,
                                         scale=float(HD) ** -0.5)
                e_half[h][off] = et

        # boundary masking: no left neighbor at t=0, no right at t=N-1
        nc.gpsimd.memset(e_half[0][0][:, 0:1], 0.0)
        nc.gpsimd.memset(e_half[1][2][:, NHALF - 1:NHALF], 0.0)

        # ---- stage 1 v chunks: PE work overlapping the softmax chain ----
        for m in (6, 7, 8):
            qkvT[m] = stage1_chunk(m, "act")

        # dL[t] = v[t-1] - v[t]  (padded: dL[0] = dL[N] = 0). AV then becomes
        # attn = v + p_l*dL - p_r*shift(dL), using that p_l + p_c + p_r = 1.
        dLs = []
        for kc in range(KC):
            v = qkvT[6 + kc]
            dL = avpool.tile([P, N + 1], bf16, tag="dv", name=f"dL{kc}")
            nc.vector.memset(dL[:, 0:1], 0.0)
            nc.vector.memset(dL[:, N:N + 1], 0.0)
            nc.vector.tensor_sub(dL[:, 1:N], v[:, 0:N - 1], v[:, 1:N])
            dLs.append(dL)

        # ---- softmax over the 3 offsets (per half) ----
        p_half = [[None] * 3 for _ in range(2)]
        for h in range(2):
            e0, e1, e2 = e_half[h]
            with tc.high_priority():
                den0 = epool.tile([H, NHALF], f32, tag="e")
                nc.vector.tensor_add(den0, e0, e1)
                den = epool.tile([H, NHALF], f32, tag="e")
                nc.vector.tensor_add(den, den0, e2)
                rec = epool.tile([H, NHALF], f32, tag="e")
                nc.vector.reciprocal_approx_fast(out=rec, in_=den)
                for off in (0, 2):
                    pt = epool.tile([H, NHALF], bf16, tag="p",
                                    name=f"p{h}_{off}")
                    nc.vector.tensor_mul(pt, e_half[h][off], rec)
                    p_half[h][off] = pt

        # ---- p broadcast (PE) + AV (DVE) + interleaved projection ----
        # proj accumulates over kc as soon as attn[kc] of this half exists,
        # so the PE overlaps the AV chain. yps tiles reuse the (now idle)
        # stage-1 mm PSUM pool.
        attn = [aopool.tile([P, N], bf16, name=f"attn{kc}")
                for kc in range(KC)]
        for h in range(2):
            lo = NHALF * h
            hi = lo + NHALF
            yps = [mmpool.tile([P, NHALF], f32, tag="mm", name=f"y{m}_{h}")
                   for m in range(KC)]
            for kc in range(KC):
                v = qkvT[6 + kc]

                def bcast(off, _h=h, _kc=kc):
                    # one pb PSUM tile live at a time (pb pool: 2 slots).
                    # Lane 0: DVE multiplies straight from PSUM (1x read).
                    # Lane 1: ACT evacuates to bf16 SBUF first so the DVE
                    # multiply runs in 2x mode - balances ACT vs DVE load.
                    pbps = pbpool.tile([P, NHALF], f32, tag="pb",
                                       name=f"pb{_kc}_{off}_{_h}")
                    nc.tensor.matmul(
                        pbps,
                        lhsT=ind6T[:, P * _kc:P * (_kc + 1)],
                        rhs=p_half[_h][off],
                        start=True, stop=True,
                    )
                    if _h == 0:
                        return pbps
                    pbs = avpool.tile([P, NHALF], bf16, tag="pbs")
                    nc.scalar.copy(pbs, pbps)
                    return pbs

                dL = dLs[kc]
                pb = bcast(0)
                m1 = avpool.tile([P, NHALF], bf16, tag="m")
                nc.vector.tensor_mul(m1, pb, dL[:, lo:hi])
                pb = bcast(2)
                m2 = avpool.tile([P, NHALF], bf16, tag="m")
                nc.vector.tensor_mul(m2, pb, dL[:, lo + 1:hi + 1])
                s12 = avpool.tile([P, NHALF], bf16, tag="m")
                nc.vector.tensor_sub(s12, m1, m2)
                nc.vector.tensor_add(half(attn[kc], h), s12, v[:, lo:hi])

                for m in range(KC):
                    nc.tensor.matmul(
                        yps[m],
                        lhsT=w_proj[kc][:, P * m:P * (m + 1)],
                        rhs=half(attn[kc], h),
                        start=(kc == 0), stop=False,
                    )

            for m in range(KC):
                nc.tensor.matmul(
                    yps[m],
                    lhsT=bias[:, P * m:P * (m + 1)],
                    rhs=half(ones, h),
                    start=False, stop=True,
                )
                yt = ypool.tile([P, NHALF], bf16, tag="y")
                nc.scalar.copy(yt, yps[m])
                nc.sync.dma_start(
                    out=d_yT[P * m:P * (m + 1), NHALF * h:NHALF * (h + 1)],
                    in_=yt)

    nc.compile()
    return nc


def _host_inputs(x, qkv_w, proj_w, proj_b):
    import ml_dtypes
    bf = ml_dtypes.bfloat16

    qkv_wT = np.ascontiguousarray(qkv_w.astype(np.float32).T).astype(bf)
    proj_wT = np.ascontiguousarray(proj_w.astype(np.float32).T).astype(bf)
    bias = proj_b.astype(np.float32).reshape(1, C).astype(bf)
    # head indicator: row p of chunk kc belongs to head 2*kc + p//64
    ind6 = np.zeros((P, 6 * KC), np.float32)
    ind6T = np.zeros((H, P * KC), np.float32)
    for kc in range(KC):
        for p in range(P):
            ind6[p, 6 * kc + 2 * kc + p // HD] = 1.0
            ind6T[2 * kc + p // HD, P * kc + p] = 1.0
    shared = {
        "qkv_wT": qkv_wT,
        "proj_wT": proj_wT,
        "proj_b": bias,
        "ind6": ind6.astype(bf),
        "ind6T": ind6T.astype(bf),
        "ones": np.ones((1, N), bf),
    }
    in_maps = []
    for b in range(B):
        m = dict(shared)
        m["xT"] = np.ascontiguousarray(x[b].astype(np.float32).T).astype(bf)
        in_maps.append(m)
    return in_maps


def kernel(x, qkv_w, proj_w, proj_b, _trace=False):
    from concourse import bass_utils

    x = np.asarray(x)
    if "nc" not in _cached:
        _cached["nc"] = _build_nc()
    nc = _cached["nc"]
    in_maps = _host_inputs(x, np.asarray(qkv_w), np.asarray(proj_w),
                           np.asarray(proj_b))
    res = bass_utils.run_bass_kernel_spmd(
        nc, in_maps, core_ids=list(range(NCORES)), trace=_trace)
    out = np.empty((B, N, C), np.float32)
    for b in range(B):
        out[b] = res.results[b]["yT"].astype(np.float32).T
    if _trace:
        _cached["last_result"] = res
    return out



# revision 13
# speedup vs baseline: 1.0319x; 1.0319x over previous
"""Local (banded, window=3) attention TRN2 kernel, v3.

Full-input contract: kernel(**inputs) takes the complete tensors
  x [8, 1024, 384], qkv_w [1152, 384], proj_w [384, 384], proj_b [384]
and returns the full output [8, 1024, 384].

Sharding: data-parallel over batch B=8 -> one batch element per NeuronCore.

Per-core algorithm (bf16 data, fp32 PSUM, feature-major [C, N] layout):
  dx[t] = x[t-1] - x[t]  (DVE, once per contraction chunk)
  qT, vT = W_{q,v} @ x.T;  dkT = W_k @ dx.T  (PE; dk[t] = k[t-1]-k[t]
    directly, no post-hoc shift-subtract needed)
  score DIFFERENCES (softmax is shift-invariant):
    s_l - s_c = q[t].dk[t],  s_r - s_c = -q[t].dk[t+1]
    products on DVE; head-reduction via PE matmul against +-scale
    indicator weights (sign and 1/sqrt(hd) folded into the indicator),
    both offsets accumulated into ONE [12, 512] PSUM region per half
    (h0 at partition 0, h1 at partition 32 of the same bank).
  e = exp(s_psum) on ACT (one instr per half); boundary cols masked.
  p_l = e_l / (1 + e_l + e_r), p_r likewise  (DVE divide, bf16)
  pb = broadcast p to the 64 feature rows of its head via PE indicator
  attn = v + pb_l * dL[t] - pb_r * dL[t+1],  dL[t] = v[t-1] - v[t]
    (muls on DVE; the subtract+add run on Pool as scalar_tensor_tensor)
  y = proj_w @ attn (PE), bias folded into the ACT PSUM->SBUF evacuation
    (activation Identity with per-partition bias vector).

Engine split: PE matmuls only (~49k cycles); DVE dx/prods/softmax/dL/AV
muls + v-h0 evac; ACT q,dk,v-h1 evac + exp + pb-h1 evac + y evac; Pool
memsets/masks + AV combine; input DMAs batched on the sync queue.
"""

import numpy as np

B, N, C = 8, 1024, 384
H, HD = 6, 64
CQKV = 3 * C  # 1152
NCORES = 8
P = 128
NHALF = N // 2  # 512
KC = C // P  # 3 contraction chunks
SCALE = float(HD) ** -0.5

_cached = {}


def _build_nc():
    import contextlib

    import concourse.bacc as bacc
    import concourse.tile as tile
    from concourse import mybir

    f32 = mybir.dt.float32
    bf16 = mybir.dt.bfloat16
    AF = mybir.ActivationFunctionType
    DIV = mybir.AluOpType.divide
    MUL = mybir.AluOpType.mult
    ADD = mybir.AluOpType.add
    BYP = mybir.AluOpType.bypass

    nc = bacc.Bacc("TRN2", target_bir_lowering=False, debug=False,
                   num_devices=NCORES)

    # host-packed inputs (see _host_inputs for layouts)
    d_x0 = nc.dram_tensor("x0", [P, N], bf16, kind="ExternalInput").ap()
    d_x12 = nc.dram_tensor("x12", [P, 2 * N], bf16,
                           kind="ExternalInput").ap()
    d_wqk = nc.dram_tensor("wqk", [P, 3 * 768], bf16,
                           kind="ExternalInput").ap()
    d_wvp = nc.dram_tensor("wvp", [P, 3 * 768], bf16,
                           kind="ExternalInput").ap()
    d_aux = nc.dram_tensor("aux", [P, 807], bf16,
                           kind="ExternalInput").ap()
    d_yT = nc.dram_tensor("yT", [C, N], bf16, kind="ExternalOutput").ap()

    with tile.TileContext(nc) as tc, contextlib.ExitStack() as ctx:
        wpool = ctx.enter_context(tc.tile_pool(name="w", bufs=1))
        qkvpool = ctx.enter_context(tc.tile_pool(name="qkv", bufs=1))
        prodpool = ctx.enter_context(tc.tile_pool(name="prod", bufs=1))
        avpool = ctx.enter_context(tc.tile_pool(name="av", bufs=1))
        epool = ctx.enter_context(tc.tile_pool(name="e", bufs=1))
        ypool = ctx.enter_context(tc.tile_pool(name="y", bufs=4))
        # PSUM budget (8 banks of 512 fp32):
        #   mm  [128,512] x 4  (stage-1 qkv, reused for proj y)
        #   s   [128,512] x 1  (scores: h0 rows 0:12, h1 rows 32:44)
        #   pb  [128,512] x 3  (p-broadcast)
        mmpool = ctx.enter_context(
            tc.tile_pool(name="mm", bufs=4, space="PSUM"))
        spool = ctx.enter_context(
            tc.tile_pool(name="s", bufs=2, space="PSUM"))
        pbpool = ctx.enter_context(
            tc.tile_pool(name="pb", bufs=2, space="PSUM"))

        # ---- input DMAs, all on the sync queue, first-needed first ----
        wqk = wpool.tile([P, 3 * 768], bf16, name="wqk")
        nc.sync.dma_start(out=wqk, in_=d_wqk)
        x0 = wpool.tile([P, N], bf16, name="x0")
        nc.sync.dma_start(out=x0, in_=d_x0)
        x12 = wpool.tile([P, 2 * N], bf16, name="x12")
        nc.sync.dma_start(out=x12, in_=d_x12)
        wvp = wpool.tile([P, 3 * 768], bf16, name="wvp")
        nc.sync.dma_start(out=wvp, in_=d_wvp)
        aux = wpool.tile([P, 807], bf16, name="aux")
        nc.sync.dma_start(out=aux[:, 0:39], in_=d_aux[:, 0:39])
        nc.sync.dma_start(out=aux[0:6, 39:807], in_=d_aux[0:6, 39:807])

        def xv(kc):
            if kc == 0:
                return x0
            return x12[:, N * (kc - 1):N * kc]

        def w_s1(m, kc):
            # stage-1 lhsT for output chunk m (0-2 q, 3-5 k->dk, 6-8 v)
            if m < 3:
                c0 = 768 * kc + P * m
                return wqk[:, c0:c0 + P]
            if m < 6:
                c0 = 768 * kc + 384 + P * (m - 3)
                return wqk[:, c0:c0 + P]
            c0 = 768 * kc + P * (m - 6)
            return wvp[:, c0:c0 + P]

        def w_pj(mp, kc):
            c0 = 768 * kc + 384 + P * mp
            return wvp[:, c0:c0 + P]

        def ind_s(kc, off):
            c0 = 6 * (2 * kc + off)
            return aux[:, c0:c0 + 6]

        def bias_col(m):
            return aux[:, 36 + m:37 + m]

        def ind_b(kc, off):
            c0 = 39 + P * (2 * kc + off)
            return aux[0:6, c0:c0 + P]

        # dx[kc][t] = x[t-1] - x[t]; col 0 zeroed (masked later anyway)
        dx = [prodpool.tile([P, N], bf16, name=f"dx{kc}")
              for kc in range(KC)]
        # dL tiles exist up-front so their boundary memsets can run early
        dL = [avpool.tile([P, N + 1], bf16, name=f"dL{kc}")
              for kc in range(KC)]
        for kc in range(KC):
            nc.gpsimd.memset(dx[kc][:, 0:1], 0.0)
            nc.gpsimd.memset(dL[kc][:, 0:1], 0.0)
            nc.gpsimd.memset(dL[kc][:, N:N + 1], 0.0)
        for kc in range(KC):
            xk = xv(kc)
            nc.vector.tensor_sub(dx[kc][:, 1:N], xk[:, 0:N - 1],
                                 xk[:, 1:N])

        # ---- stage 1: q (0-2), dk (3-5), v (6-8) ----
        qkvT = [None] * 9

        def stage1(m):
            qt = qkvpool.tile([P, N], bf16, name=f"qkvT{m}")
            for h in range(2):
                ps = mmpool.tile([P, NHALF], f32, tag="mm")
                for kc in range(KC):
                    rhs = dx[kc] if 3 <= m < 6 else xv(kc)
                    nc.tensor.matmul(
                        ps, lhsT=w_s1(m, kc),
                        rhs=rhs[:, NHALF * h:NHALF * (h + 1)],
                        start=(kc == 0), stop=(kc == KC - 1))
                dst = qt[:, NHALF * h:NHALF * (h + 1)]
                # v-h0 evac on DVE, everything else on ACT
                if m >= 6 and h == 0:
                    nc.vector.tensor_copy(dst, ps)
                else:
                    nc.scalar.copy(dst, ps)
            qkvT[m] = qt

        prods = [[None, None] for _ in range(KC)]  # [kc][l, r]

        def post_k(kc):
            q = qkvT[kc]
            dk = qkvT[3 + kc]
            pl = prodpool.tile([P, N], bf16, name=f"pl{kc}")
            nc.vector.tensor_mul(pl, q, dk)
            pr = prodpool.tile([P, N], bf16, name=f"pr{kc}")
            # col N-1 left unwritten: garbage flows into e_r[N-1], which
            # is masked to 0 right after the exp
            nc.vector.tensor_mul(pr[:, 0:N - 1], q[:, 0:N - 1],
                                 dk[:, 1:N])
            prods[kc][0] = pl
            prods[kc][1] = pr

        for m in (0, 3):
            stage1(m)
        post_k(0)
        for m in (1, 4):
            stage1(m)
        post_k(1)
        for m in (2, 5):
            stage1(m)
        post_k(2)
        for m in (6, 7, 8):
            stage1(m)
        # dL[kc][t] = v[t-1] - v[t] (cols 0 and N are pre-zeroed)
        for kc in range(KC):
            v = qkvT[6 + kc]
            nc.vector.tensor_sub(dL[kc][:, 1:N], v[:, 0:N - 1], v[:, 1:N])

        p_all = [[None, None], [None, None]]  # [h][off]

        def scores(h):
            s_ps = spool.tile([P, NHALF], f32, tag="s", name=f"s_ps{h}")
            es = []
            for off in range(2):
                base = 32 * off  # aligned PSUM partition groups
                sub = s_ps[base:base + 6, :]
                for kc in range(KC):
                    nc.tensor.matmul(
                        sub, lhsT=ind_s(kc, off),
                        rhs=prods[kc][off][:, NHALF * h:NHALF * (h + 1)],
                        start=(kc == 0), stop=(kc == KC - 1))
                e = epool.tile([6, NHALF], bf16, tag="e", bufs=8,
                               name=f"e{h}_{off}")
                with tc.high_priority():
                    nc.scalar.activation(e, sub, AF.Exp)
                es.append(e)
            el, er = es
            # boundary: no left neighbor at t=0, no right at t=N-1
            if h == 0:
                nc.gpsimd.memset(el[:, 0:1], 0.0)
            else:
                nc.gpsimd.memset(er[:, NHALF - 1:NHALF], 0.0)
            with tc.high_priority():
                den0 = epool.tile([6, NHALF], bf16, tag="e", bufs=8)
                nc.vector.tensor_add(den0, el, er)
                den1 = epool.tile([6, NHALF], f32, tag="ef", bufs=4)
                nc.vector.tensor_scalar_add(den1, den0, 1.0)
                rec = epool.tile([6, NHALF], f32, tag="ef", bufs=4)
                nc.vector.reciprocal_approx_fast(out=rec, in_=den1)
                pl = epool.tile([6, NHALF], bf16, tag="p", bufs=4,
                                name=f"p{h}_l")
                nc.vector.tensor_mul(pl, el, rec)
                pr = epool.tile([6, NHALF], bf16, tag="p", bufs=4,
                                name=f"p{h}_r")
                nc.vector.tensor_mul(pr, er, rec)
            p_all[h] = [pl, pr]

        scores(0)
        scores(1)

        # ---- p broadcast (PE) + AV + projection ----
        attn = [avpool.tile([P, N], bf16, name=f"attn{kc}")
                for kc in range(KC)]

        pbs = {}
        for h in range(2):
            for kc in range(KC):
                for off in range(2):
                    pb = pbpool.tile([P, NHALF], f32, tag="pb",
                                     name=f"pb{h}_{kc}_{off}")
                    nc.tensor.matmul(pb, lhsT=ind_b(kc, off),
                                     rhs=p_all[h][off], start=True,
                                     stop=True)
                    pbs[(h, kc, off)] = pb

        def av(h, kc):
            lo = NHALF * h
            hi = lo + NHALF
            pbl, pbr = pbs[(h, kc, 0)], pbs[(h, kc, 1)]
            if h == 1:
                # ACT evacuates to bf16 so the DVE muls run in 2x mode
                pbls = avpool.tile([P, NHALF], bf16, tag="pbs", bufs=4)
                nc.scalar.copy(pbls, pbl)
                pbl = pbls
                pbrs = avpool.tile([P, NHALF], bf16, tag="pbs", bufs=4)
                nc.scalar.copy(pbrs, pbr)
                pbr = pbrs
            m1 = avpool.tile([P, NHALF], bf16, tag="m", bufs=6)
            nc.vector.tensor_mul(m1, pbl, dL[kc][:, lo:hi])
            m2 = avpool.tile([P, NHALF], bf16, tag="m", bufs=6)
            nc.vector.tensor_mul(m2, pbr, dL[kc][:, lo + 1:hi + 1])
            # Pool adds: the r-broadcast indicator is negated, so
            # s12 = m1 + m2 and attn = s12 + v are plain tensor_adds
            s12 = avpool.tile([P, NHALF], bf16, tag="m", bufs=6)
            nc.gpsimd.tensor_add(s12, m1, m2)
            nc.gpsimd.tensor_add(attn[kc][:, lo:hi], s12,
                                 qkvT[6 + kc][:, lo:hi])

        for kc in range(KC):
            av(0, kc)
        for kc in range(KC):
            av(1, kc)

        for h in range(2):
            lo = NHALF * h
            hi = lo + NHALF
            yps = [mmpool.tile([P, NHALF], f32, tag="mm", name=f"y{m}_{h}")
                   for m in range(KC)]
            for kc in range(KC):
                for mp in range(KC):
                    nc.tensor.matmul(
                        yps[mp], lhsT=w_pj(mp, kc),
                        rhs=attn[kc][:, lo:hi],
                        start=(kc == 0), stop=(kc == KC - 1))
            for mp in range(KC):
                yt = ypool.tile([P, NHALF], bf16, tag="y")
                nc.scalar.add(yt, yps[mp], bias_col(mp))
                nc.sync.dma_start(
                    out=d_yT[P * mp:P * (mp + 1), lo:hi], in_=yt)

    nc.compile()
    return nc


def _host_inputs(x, qkv_w, proj_w, proj_b):
    import ml_dtypes
    bf = ml_dtypes.bfloat16

    qkv_wT = np.ascontiguousarray(qkv_w.astype(np.float32).T)  # [384, 1152]
    proj_wT = np.ascontiguousarray(proj_w.astype(np.float32).T)  # [384, 384]

    # wqk block kc = qkv_wT[128kc:128(kc+1), 0:768]
    wqk = np.concatenate(
        [qkv_wT[P * kc:P * (kc + 1), 0:768] for kc in range(KC)],
        axis=1)
    # wvp block kc = [qkv_wT v-part | proj_wT] rows of chunk kc
    wvp = np.concatenate(
        [np.concatenate([qkv_wT[P * kc:P * (kc + 1), 768:1152],
                         proj_wT[P * kc:P * (kc + 1), :]], axis=1)
         for kc in range(KC)],
        axis=1)

    aux = np.zeros((P, 807), np.float32)
    for kc in range(KC):
        for p in range(P):
            h = 2 * kc + p // HD
            # ind_s: block (kc, l) = +scale, block (kc, r) = -scale
            # (sign of the r score difference)
            aux[p, 6 * (2 * kc + 0) + h] = SCALE
            aux[p, 6 * (2 * kc + 1) + h] = -SCALE
    for m in range(KC):
        aux[:, 36 + m] = proj_b[P * m:P * (m + 1)].astype(np.float32)
    for kc in range(KC):
        for i in range(P):
            h = 2 * kc + i // HD
            aux[h, 39 + P * (2 * kc + 0) + i] = 1.0
            aux[h, 39 + P * (2 * kc + 1) + i] = -1.0

    shared = {
        "wqk": wqk.astype(bf),
        "wvp": wvp.astype(bf),
        "aux": aux.astype(bf),
    }
    in_maps = []
    for b in range(B):
        m = dict(shared)
        xT = np.ascontiguousarray(x[b].astype(np.float32).T)  # [384, 1024]
        m["x0"] = xT[0:P, :].astype(bf)
        m["x12"] = np.concatenate([xT[P:2 * P, :], xT[2 * P:3 * P, :]],
                                  axis=1).astype(bf)
        in_maps.append(m)
    return in_maps


def kernel(x, qkv_w, proj_w, proj_b, _trace=False):
    from concourse import bass_utils

    x = np.asarray(x)
    if "nc" not in _cached:
        _cached["nc"] = _build_nc()
    nc = _cached["nc"]
    in_maps = _host_inputs(x, np.asarray(qkv_w), np.asarray(proj_w),
                           np.asarray(proj_b))
    res = bass_utils.run_bass_kernel_spmd(
        nc, in_maps, core_ids=list(range(NCORES)), trace=_trace)
    out = np.empty((B, N, C), np.float32)
    for b in range(B):
        out[b] = res.results[b]["yT"].astype(np.float32).T
    if _trace:
        _cached["last_result"] = res
    return out
